# revision 2
# baseline (speedup 1.0000x reference)
"""Distributed multi-head attention kernel for trn2 (8 NeuronCores), v2.

Problem: B=2, N=4096, C=768, H=8 heads, Dh=96.
    qkv = x @ w_qkv ; per-head softmax(q k^T / sqrt(Dh)) v ; out @ w_proj + b_proj

Sharding (data parallel on B, tensor parallel on heads):
    core i -> batch b = i//4, heads (2*(i%4), 2*(i%4)+1)

v2 restructure vs v1 (both flash-style, fully on-chip):
  - PV matmul runs in o[q-part, d] orientation (lhsT = p-chunk, rhs = v):
    97-column streams per (ktile, q128) instead of 512-column streams per
    ktile -- 24% fewer PE cycles in PV. The softmax denominator arrives in
    PSUM column 96 (ones-column on v), per-PARTITION now, so normalization
    is a cheap DVE reciprocal + tensor_scalar broadcast; the K=1
    outer-product broadcast matmuls of v1 are gone.
  - o[q,d] -> oT[d,q] for the projection is done by the DMA XBAR transpose
    (dma_start_transpose on the SP queue), off the critical PE/ACT engines.
  - head 1's QKV matmul units are interleaved into head 0's attention
    stream, hiding ~25us of PE work under the ACT-bound attention phase.

Math notes (unchanged from v1):
  - scores ~ N(0,1) after the Dh^-0.5 scale (folded into w_q on host), so the
    softmax max-subtraction is skipped (exp stays < ~1e3, safely inside f32).
  - compute dtype is bf16 on the TensorEngine (f32 PSUM accumulation); the
    ReduceScatter payload is bf16; rel err vs the f32 reference ~5e-3.
"""

import numpy as np
import ml_dtypes

import concourse.bass as bass
import concourse.tile as tile
from concourse import mybir
from concourse.bass_utils import run_bass_kernel_spmd

# ---------------- problem constants (hardcoded per spec) ----------------
B, N, C, H, DH = 2, 4096, 768, 8, 96
HEADS_PER_CORE = 2
HD = HEADS_PER_CORE * DH  # 192
N_CORES = 8
GROUPS = [[0, 1, 2, 3], [4, 5, 6, 7]]
QR = 512  # query rows per o-accumulation group (4 PSUM banks of [128,97])
N_QR = N // QR  # 8
KC = 128  # key chunk (contraction tile for PV)
N_KC = N // KC  # 32

F32 = mybir.dt.float32
BF16 = mybir.dt.bfloat16
BF16_NP = ml_dtypes.bfloat16

_DEBUG_NO_RS = False  # replace ReduceScatter with a local copy (debug only)
_INTERLEAVE_QKV1 = True  # interleave head-1 QKV into head-0 attention


def _hoist_waits(nc):
    """The staged walrus build rejects instructions carrying more than one
    inline sync wait ("Too many sync wait commands"). Move every instruction's
    on_wait list into standalone EventSemaphore instructions immediately
    before it (same engine, same block) -- the encoding raw-bass wait_ge uses."""
    ctr = 0
    for bb in nc.main_func.blocks:
        out = []
        changed = False
        for ins in bb.instructions:
            si = getattr(ins, "sync_info", None)
            if si is not None and si.on_wait:
                for w in si.on_wait:
                    ctr += 1
                    out.append(
                        mybir.InstEventSemaphore(
                            name=f"hoistw-{ctr}",
                            opcode="EventSemaphore",
                            engine=ins.engine,
                            ins=[],
                            outs=[],
                            sync_info=mybir.SyncInfo(on_wait=[w], on_update=[]),
                        )
                    )
                ins.sync_info = mybir.SyncInfo(on_wait=[], on_update=si.on_update)
                changed = True
            out.append(ins)
        if changed:
            try:
                bb.instructions = out
            except Exception:
                bb.instructions.clear()
                bb.instructions.extend(out)
    return nc


def build(reps: int = 1):
    nc = bass.Bass()

    xT = nc.declare_dram_parameter("xT", [C, N], BF16, isOutput=False)
    wq = nc.declare_dram_parameter("wq", [C, HD], BF16, isOutput=False)
    wk = nc.declare_dram_parameter("wk", [C, HD], BF16, isOutput=False)
    wv = nc.declare_dram_parameter("wv", [C, HD], BF16, isOutput=False)
    wp = nc.declare_dram_parameter("wp", [HD, C], BF16, isOutput=False)
    bias = nc.declare_dram_parameter("bias", [C], F32, isOutput=False)
    out_ext = nc.declare_dram_parameter("out", [N // 4, C], F32, isOutput=True)

    NCC = C // 128  # 6 contraction chunks over C

    with tile.TileContext(nc) as tc:
        with (
            tc.tile_pool(name="dram", bufs=4, space="DRAM") as dram,
            tc.tile_pool(name="const", bufs=1) as const,
            tc.tile_pool(name="ps", bufs=3, space="PSUM") as ps,
            tc.tile_pool(name="op", bufs=2, space="PSUM") as op,
            tc.tile_pool(name="pp", bufs=4) as pp,
            tc.tile_pool(name="onp", bufs=3) as onp,
            tc.tile_pool(name="misc", bufs=4) as misc,
            tc.tile_pool(name="yb", bufs=3) as ybp,
        ):
            for _rep in range(reps):
                # ---------------- constant loads ----------------
                # weights on the SP ring, xT chunks on the ACT ring so the first
                # QKV matmul only gates on wq + xT chunk 0 (parallel rings)
                wq_sb = const.tile([128, NCC, HD], BF16)
                nc.sync.dma_start(wq_sb, wq.rearrange("(co p) m -> p co m", p=128))
                wk_sb = const.tile([128, NCC, HD], BF16)
                nc.sync.dma_start(wk_sb, wk.rearrange("(co p) m -> p co m", p=128))
                wv_sb = const.tile([128, NCC, HD], BF16)
                nc.sync.dma_start(wv_sb, wv.rearrange("(co p) m -> p co m", p=128))
                wp_sb = const.tile([96, HEADS_PER_CORE, C], BF16)
                nc.sync.dma_start(wp_sb, wp.rearrange("(h p) m -> p h m", p=96))
                xT_sb = const.tile([128, NCC, N], BF16)
                # n-major quarter slices: the first q/k units need only the
                # first 1024 cols of every c-chunk (~1.5MB), not the full xT
                for nq in range(4):
                    for c in range(NCC):
                        eng = nc.scalar if c % 2 == 0 else nc.sync
                        eng.dma_start(
                            xT_sb[:, c, nq * 1024 : (nq + 1) * 1024],
                            xT[c * 128 : (c + 1) * 128, nq * 1024 : (nq + 1) * 1024],
                        )
                # bias prefill: broadcast b_proj over all output rows now; the
                # ReduceScatter result is accumulated on top at the end
                nc.gpsimd.dma_start(
                    out_ext[:, :],
                    bass.AP(
                        tensor=bias.ap().tensor, offset=0, ap=[[0, N // 4], [1, C]]
                    ),
                )

                # ---------------- QKV ----------------
                # q^T, k^T in [Dh, N] layout (streamed); v in [N, Dh] layout with an
                # appended ones column for the softmax denominator.
                qT_sb = [const.tile([96, N], BF16, name=f"qT{h}") for h in range(2)]
                kT_sb = [const.tile([96, N], BF16, name=f"kT{h}") for h in range(2)]
                vp_sb = [const.tile([128, N_KC, 97], BF16, name=f"vp{h}") for h in range(2)]
                for h in range(2):
                    nc.vector.memset(vp_sb[h][:, :, 96:97], 1.0)
                # oT layout for the projection: [128, 32, 128] == [Dh(pad), n/128, 128]
                # (d padded 96->128 so the XBAR DMA transpose tiles align; proj
                # reads partitions 0:96 only)
                onT_sb = [
                    const.tile([128, N // 128, 128], BF16, name=f"onT{h}")
                    for h in range(2)
                ]

                def qkv_units(h):
                    """Emit-closures for head h's QKV, ~2-6k PE cycles each."""
                    units = []

                    def qk_unit(w_sb, dst, n2):
                        def emit():
                            # two 512-col n-chunks share one [128,1024] psum slot
                            acc = ps.tile([128, 1024], F32, tag="ps")
                            for half in range(2):
                                n = 2 * n2 + half
                                for c in range(NCC):
                                    nc.tensor.matmul(
                                        acc[:96, half * 512 : (half + 1) * 512],
                                        lhsT=w_sb[:, c, h * 96 : (h + 1) * 96],
                                        rhs=xT_sb[:, c, n * 512 : (n + 1) * 512],
                                        start=(c == 0),
                                        stop=(c == NCC - 1),
                                    )
                            nc.vector.tensor_copy(
                                out=dst[:, n2 * 1024 : (n2 + 1) * 1024],
                                in_=acc[:96, :1024],
                            )

                        return emit

                    def v_unit(n2):
                        def emit():
                            # two [128,96] n-chunks of v in one [128,1024] slot
                            vacc = ps.tile([128, 1024], F32, tag="ps")
                            for half in range(2):
                                n = 2 * n2 + half
                                for c in range(NCC):
                                    nc.tensor.matmul(
                                        vacc[:, half * 512 : half * 512 + 96],
                                        lhsT=xT_sb[:, c, n * 128 : (n + 1) * 128],
                                        rhs=wv_sb[:, c, h * 96 : (h + 1) * 96],
                                        start=(c == 0),
                                        stop=(c == NCC - 1),
                                    )
                            for half in range(2):
                                nc.vector.tensor_copy(
                                    out=vp_sb[h][:, 2 * n2 + half, 0:96],
                                    in_=vacc[:, half * 512 : half * 512 + 96],
                                )

                        return emit

                    # ordered by xT n-quarter arrival: [q,k,v*4] per quarter
                    for n2 in range(N // 1024):
                        units.append(qk_unit(wq_sb, qT_sb[h], n2))
                        units.append(qk_unit(wk_sb, kT_sb[h], n2))
                        for nv in range(4 * n2, 4 * n2 + 4):
                            units.append(v_unit(nv))
                    return units

                # ---------------- output combine (quartered RS) ----------------
                y_bounce = dram.tile([N, C], BF16, tag="ybounce")
                HR = N // 4  # rows per RS quarter (1024)
                SH = HR // 4  # rows per rank per quarter (256)
                rs_outs = [None] * 4

                def emit_rs(k):
                    rs_out = dram.tile([SH, C], BF16, tag="rsout")
                    if _DEBUG_NO_RS:
                        nc.sync.dma_start(rs_out[:, :], y_bounce[k * HR : k * HR + SH, :])
                    else:
                        nc.gpsimd.collective_compute(
                            "ReduceScatter",
                            mybir.AluOpType.add,
                            replica_groups=GROUPS,
                            ins=[y_bounce[k * HR : (k + 1) * HR, :].opt()],
                            outs=[rs_out.opt()],
                        )
                    rs_outs[k] = rs_out

                def emit_epilogue(k):
                    rt = ybp.tile([128, SH // 128, C], BF16, tag="rsb", bufs=2)
                    nc.sync.dma_start(
                        rt, rs_outs[k].rearrange("(o p) m -> p o m", p=128)
                    )
                    rtf = ybp.tile([128, SH // 128, C], F32, tag="rsf", bufs=2)
                    nc.vector.tensor_copy(out=rtf, in_=rt)
                    nc.gpsimd.dma_start(
                        out_ext[k * SH : (k + 1) * SH, :].rearrange(
                            "(o p) m -> p o m", p=128
                        ),
                        rtf,
                        accum_op=mybir.AluOpType.add,
                    )

                def attention(h, extra_units):
                    """Flash attention for head h over all q-ranges; pops
                    emit-closures from extra_units between score groups.
                    Each q-range's finalize (normalize/transpose/proj) is
                    deferred into the next q-range's score stream so the PE
                    priority order keeps scores (which feed the ACT engine)
                    ahead of the proj/epilogue work at range boundaries."""
                    pending = [None]
                    for qr in range(N_QR):
                        # all 4 q128-chunk accumulators share ONE psum bank;
                        # the first matmul's start=True zeroes the whole 2KB
                        # zero-region, later j's first matmuls ride on it
                        o4 = op.tile([128, 4, 97], F32, tag="o")
                        for kcp in range(N_KC // 2):
                            sp = ps.tile([128, 2, 512], F32, tag="ps")
                            for kk in range(2):
                                kc = 2 * kcp + kk
                                nc.tensor.matmul(
                                    sp[:, kk, :],
                                    lhsT=kT_sb[h][:, kc * 128 : (kc + 1) * 128],
                                    rhs=qT_sb[h][:, qr * QR : (qr + 1) * QR],
                                    start=True,
                                    stop=True,
                                )
                            p_t = pp.tile([128, 2, 512], BF16, tag="p")
                            nc.scalar.activation(
                                p_t, sp, mybir.ActivationFunctionType.Exp
                            )
                            for kk in range(2):
                                kc = 2 * kcp + kk
                                for j in range(4):
                                    nc.tensor.matmul(
                                        o4[:, j, 0:97],
                                        lhsT=p_t[:, kk, j * 128 : (j + 1) * 128],
                                        rhs=vp_sb[h][:, kc, :],
                                        start=(kcp == 0 and kk == 0 and j == 0),
                                        stop=(kcp == N_KC // 2 - 1 and kk == 1),
                                        skip_group_check=True,
                                    )
                            if kcp == 2 and pending[0] is not None:
                                pending[0]()
                                pending[0] = None
                            if extra_units and kcp in (5, 11):
                                extra_units.popleft()()

                        def finalize(qr, o4):
                            # normalize per-partition: on = o[:,:96] / o[:,96]
                            # (d padded to 128 for XBAR alignment; pad zeroed)
                            on_qr = onp.tile([128, 4, 128], BF16, tag="on")
                            nc.vector.memset(on_qr[:, :, 96:128], 0.0)
                            r4 = misc.tile([128, 4, 1], F32, tag="r")
                            nc.vector.reciprocal(r4, o4[:, :, 96:97])
                            r4_ap = r4[:, :, :]
                            r4_bc = bass.AP(
                                tensor=r4_ap.tensor,
                                offset=r4_ap.offset,
                                ap=list(r4_ap.ap[:-1]) + [[0, 96]],
                            )
                            nc.vector.tensor_tensor(
                                on_qr[:, :, 0:96],
                                o4[:, :, 0:96],
                                r4_bc,
                                mybir.AluOpType.mult,
                            )
                            # oT via DMA XBAR (SP queue; dispatch is
                            # async from the transfer, and proj is already a
                            # q-range behind via the deferred finalize)
                            nc.sync.dma_start_transpose(
                                onT_sb[h][:, qr * 4 : (qr + 1) * 4, :], on_qr
                            )

                            if h == 1:
                                # projection: y[n,:] = sum_h onT_h[:, n]^T @ wp_h
                                for ns in range(QR // 128):
                                    nchunk = qr * 4 + ns
                                    yp = ps.tile([128, 1024], F32, tag="ps")
                                    for hh in range(2):
                                        for lo, hi in [(0, 512), (512, 768)]:
                                            nc.tensor.matmul(
                                                yp[:, lo:hi],
                                                lhsT=onT_sb[hh][0:96, nchunk, :],
                                                rhs=wp_sb[:96, hh, lo:hi],
                                                start=(hh == 0),
                                                stop=(hh == 1),
                                            )
                                    y_sb = ybp.tile([128, C], BF16, tag="y")
                                    nc.vector.tensor_copy(out=y_sb, in_=yp[:, :C])
                                    # SP ring: bulk y writes (transposes
                                    # live on the ACT queue, collectives on
                                    # Pool, so nothing latency-critical queues
                                    # behind these)
                                    nc.sync.dma_start(
                                        y_bounce[
                                            nchunk * 128 : (nchunk + 1) * 128, :
                                        ],
                                        y_sb,
                                    )
                                # fire a quarter ReduceScatter every 1024 rows
                                # (each overlaps the remaining compute; only
                                # the last quarter's RS+epilogue is exposed)
                                if qr % 2 == 1:
                                    k = qr // 2
                                    emit_rs(k)
                                    emit_epilogue(k)

                        from functools import partial

                        pending[0] = partial(finalize, qr, o4)
                    pending[0]()
                    pending[0] = None

                from collections import deque

                for u in qkv_units(0):
                    u()
                head1_units = deque(qkv_units(1))
                if not _INTERLEAVE_QKV1:
                    for u in list(head1_units):
                        u()
                    head1_units.clear()
                attention(0, head1_units)
                while head1_units:
                    head1_units.popleft()()
                attention(1, None)

    _hoist_waits(nc)
    return nc


_NC_CACHE = None


def _get_nc():
    global _NC_CACHE
    if _NC_CACHE is None:
        _NC_CACHE = build()
    return _NC_CACHE


def make_in_maps(x, w_qkv, w_proj, b_proj):
    x = np.asarray(x, dtype=np.float32)
    w_qkv = np.asarray(w_qkv, dtype=np.float32).reshape(C, 3, H, DH)
    w_proj = np.asarray(w_proj, dtype=np.float32)
    b_proj = np.asarray(b_proj, dtype=np.float32)
    scale = DH ** -0.5

    xT_b = [
        np.ascontiguousarray(x[b].T).astype(BF16_NP) for b in range(B)
    ]  # [C, N] each
    in_maps = []
    for i in range(N_CORES):
        b = i // 4
        h0 = HEADS_PER_CORE * (i % 4)
        sl = slice(h0, h0 + HEADS_PER_CORE)
        wq_i = (w_qkv[:, 0, sl, :].reshape(C, HD) * scale).astype(BF16_NP)
        wk_i = w_qkv[:, 1, sl, :].reshape(C, HD).astype(BF16_NP)
        wv_i = w_qkv[:, 2, sl, :].reshape(C, HD).astype(BF16_NP)
        wp_i = np.ascontiguousarray(
            w_proj.reshape(H, DH, C)[sl].reshape(HD, C)
        ).astype(BF16_NP)
        in_maps.append(
            {
                "xT": xT_b[b],
                "wq": np.ascontiguousarray(wq_i),
                "wk": np.ascontiguousarray(wk_i),
                "wv": np.ascontiguousarray(wv_i),
                "wp": wp_i,
                "bias": b_proj,
            }
        )
    return in_maps


def assemble(results):
    # rank r of batch-group b holds, per RS quarter k, global rows
    # [k*1024 + r*256, k*1024 + (r+1)*256) in out_ext rows [k*256, (k+1)*256)
    out = np.empty((B, N, C), dtype=np.float32)
    SH = N // 16
    for i in range(N_CORES):
        b, r = i // 4, i % 4
        shard = results[i]["out"]
        for k in range(4):
            lo = k * (N // 4) + r * SH
            out[b, lo : lo + SH, :] = shard[k * SH : (k + 1) * SH]
    return out


def kernel(x, w_qkv, w_proj, b_proj):
    nc = _get_nc()
    in_maps = make_in_maps(x, w_qkv, w_proj, b_proj)
    res = run_bass_kernel_spmd(nc, in_maps, core_ids=list(range(N_CORES)))
    return assemble(res.results)


# revision 3
# speedup vs baseline: 1.0840x; 1.0840x over previous
"""Distributed multi-head attention kernel for trn2 (8 NeuronCores), v2.

Problem: B=2, N=4096, C=768, H=8 heads, Dh=96.
    qkv = x @ w_qkv ; per-head softmax(q k^T / sqrt(Dh)) v ; out @ w_proj + b_proj

Sharding (data parallel on B, tensor parallel on heads):
    core i -> batch b = i//4, heads (2*(i%4), 2*(i%4)+1)

v2 restructure vs v1 (both flash-style, fully on-chip):
  - PV matmul runs in o[q-part, d] orientation (lhsT = p-chunk, rhs = v):
    97-column streams per (ktile, q128) instead of 512-column streams per
    ktile -- 24% fewer PE cycles in PV. The softmax denominator arrives in
    PSUM column 96 (ones-column on v), per-PARTITION now, so normalization
    is a cheap DVE reciprocal + tensor_scalar broadcast; the K=1
    outer-product broadcast matmuls of v1 are gone.
  - o[q,d] -> oT[d,q] for the projection is done by the DMA XBAR transpose
    (dma_start_transpose on the SP queue), off the critical PE/ACT engines.
  - head 1's QKV matmul units are interleaved into head 0's attention
    stream, hiding ~25us of PE work under the ACT-bound attention phase.

Math notes (unchanged from v1):
  - scores ~ N(0,1) after the Dh^-0.5 scale (folded into w_q on host), so the
    softmax max-subtraction is skipped (exp stays < ~1e3, safely inside f32).
  - compute dtype is bf16 on the TensorEngine (f32 PSUM accumulation); the
    ReduceScatter payload is bf16; rel err vs the f32 reference ~5e-3.
"""

import numpy as np
import ml_dtypes

import concourse.bass as bass
import concourse.tile as tile
from concourse import mybir
from concourse.bass_utils import run_bass_kernel_spmd

# ---------------- problem constants (hardcoded per spec) ----------------
B, N, C, H, DH = 2, 4096, 768, 8, 96
HEADS_PER_CORE = 2
HD = HEADS_PER_CORE * DH  # 192
N_CORES = 8
GROUPS = [[0, 1, 2, 3], [4, 5, 6, 7]]
QR = 512  # query rows per o-accumulation group (4 PSUM banks of [128,97])
N_QR = N // QR  # 8
KC = 128  # key chunk (contraction tile for PV)
N_KC = N // KC  # 32

F32 = mybir.dt.float32
BF16 = mybir.dt.bfloat16
BF16_NP = ml_dtypes.bfloat16

_DEBUG_NO_RS = False  # replace ReduceScatter with a local copy (debug only)
_INTERLEAVE_QKV1 = True  # interleave head-1 QKV into head-0 attention


def _hoist_waits(nc):
    """The staged walrus build rejects instructions carrying more than one
    inline sync wait ("Too many sync wait commands"). Move every instruction's
    on_wait list into standalone EventSemaphore instructions immediately
    before it (same engine, same block) -- the encoding raw-bass wait_ge uses."""
    ctr = 0
    for bb in nc.main_func.blocks:
        out = []
        changed = False
        for ins in bb.instructions:
            si = getattr(ins, "sync_info", None)
            if si is not None and si.on_wait:
                for w in si.on_wait:
                    ctr += 1
                    out.append(
                        mybir.InstEventSemaphore(
                            name=f"hoistw-{ctr}",
                            opcode="EventSemaphore",
                            engine=ins.engine,
                            ins=[],
                            outs=[],
                            sync_info=mybir.SyncInfo(on_wait=[w], on_update=[]),
                        )
                    )
                ins.sync_info = mybir.SyncInfo(on_wait=[], on_update=si.on_update)
                changed = True
            out.append(ins)
        if changed:
            try:
                bb.instructions = out
            except Exception:
                bb.instructions.clear()
                bb.instructions.extend(out)
    return nc


def build(reps: int = 1):
    nc = bass.Bass()

    xT = nc.declare_dram_parameter("xT", [C, N], BF16, isOutput=False)
    wq = nc.declare_dram_parameter("wq", [C, HD], BF16, isOutput=False)
    wk = nc.declare_dram_parameter("wk", [C, HD], BF16, isOutput=False)
    wv = nc.declare_dram_parameter("wv", [C, HD], BF16, isOutput=False)
    wp = nc.declare_dram_parameter("wp", [HD, C], BF16, isOutput=False)
    bias = nc.declare_dram_parameter("bias", [C], F32, isOutput=False)
    out_ext = nc.declare_dram_parameter("out", [N // 4, C], F32, isOutput=True)

    NCC = C // 128  # 6 contraction chunks over C

    with tile.TileContext(nc) as tc:
        with (
            tc.tile_pool(name="dram", bufs=4, space="DRAM") as dram,
            tc.tile_pool(name="const", bufs=1) as const,
            tc.tile_pool(name="ps", bufs=3, space="PSUM") as ps,
            tc.tile_pool(name="op", bufs=2, space="PSUM") as op,
            tc.tile_pool(name="pp", bufs=4) as pp,
            tc.tile_pool(name="onp", bufs=3) as onp,
            tc.tile_pool(name="misc", bufs=4) as misc,
            tc.tile_pool(name="yb", bufs=3) as ybp,
        ):
            for _rep in range(reps):
                # ---------------- constant loads ----------------
                # weights on the SP ring, xT chunks on the ACT ring so the first
                # QKV matmul only gates on wq + xT chunk 0 (parallel rings)
                wq_sb = const.tile([128, NCC, HD], BF16)
                nc.sync.dma_start(wq_sb, wq.rearrange("(co p) m -> p co m", p=128))
                wk_sb = const.tile([128, NCC, HD], BF16)
                nc.sync.dma_start(wk_sb, wk.rearrange("(co p) m -> p co m", p=128))
                wv_sb = const.tile([128, NCC, HD], BF16)
                nc.sync.dma_start(wv_sb, wv.rearrange("(co p) m -> p co m", p=128))
                wp_sb = const.tile([96, HEADS_PER_CORE, C], BF16)
                nc.sync.dma_start(wp_sb, wp.rearrange("(h p) m -> p h m", p=96))
                xT_sb = const.tile([128, NCC, N], BF16)
                # n-major quarter slices: the first q/k units need only the
                # first 1024 cols of every c-chunk (~1.5MB), not the full xT
                for nq in range(4):
                    for c in range(NCC):
                        eng = nc.scalar if c % 2 == 0 else nc.sync
                        eng.dma_start(
                            xT_sb[:, c, nq * 1024 : (nq + 1) * 1024],
                            xT[c * 128 : (c + 1) * 128, nq * 1024 : (nq + 1) * 1024],
                        )
                # bias prefill: broadcast b_proj over all output rows; only
                # needed by the epilogue accumulate, so it queues on the Pool
                # ring AFTER the xT slices it would otherwise delay
                nc.gpsimd.dma_start(
                    out_ext[:, :],
                    bass.AP(
                        tensor=bias.ap().tensor, offset=0, ap=[[0, N // 4], [1, C]]
                    ),
                )

                # ---------------- QKV ----------------
                # q^T, k^T in [Dh, N] layout (streamed); v in [N, Dh] layout with an
                # appended ones column for the softmax denominator.
                qT_sb = [const.tile([96, N], BF16, name=f"qT{h}") for h in range(2)]
                kT_sb = [const.tile([96, N], BF16, name=f"kT{h}") for h in range(2)]
                vp_sb = [const.tile([128, N_KC, 97], BF16, name=f"vp{h}") for h in range(2)]
                for h in range(2):
                    nc.vector.memset(vp_sb[h][:, :, 96:97], 1.0)
                # oT layout for the projection: [128, 32, 128] == [Dh(pad), n/128, 128]
                # (d padded 96->128 so the XBAR DMA transpose tiles align; proj
                # reads partitions 0:96 only)
                onT_sb = [
                    const.tile([128, N // 128, 128], BF16, name=f"onT{h}")
                    for h in range(2)
                ]

                def qkv_units(h):
                    """Emit-closures for head h's QKV, ~2-6k PE cycles each."""
                    units = []

                    def qk_unit(w_sb, dst, n2):
                        def emit():
                            # two 512-col n-chunks share one [128,1024] psum slot
                            acc = ps.tile([128, 1024], F32, tag="ps")
                            for half in range(2):
                                n = 2 * n2 + half
                                for c in range(NCC):
                                    nc.tensor.matmul(
                                        acc[:96, half * 512 : (half + 1) * 512],
                                        lhsT=w_sb[:, c, h * 96 : (h + 1) * 96],
                                        rhs=xT_sb[:, c, n * 512 : (n + 1) * 512],
                                        start=(c == 0),
                                        stop=(c == NCC - 1),
                                    )
                            nc.vector.tensor_copy(
                                out=dst[:, n2 * 1024 : (n2 + 1) * 1024],
                                in_=acc[:96, :1024],
                            )

                        return emit

                    def v_unit(n2):
                        def emit():
                            # two [128,96] n-chunks of v in one [128,1024] slot
                            vacc = ps.tile([128, 1024], F32, tag="ps")
                            for half in range(2):
                                n = 2 * n2 + half
                                for c in range(NCC):
                                    nc.tensor.matmul(
                                        vacc[:, half * 512 : half * 512 + 96],
                                        lhsT=xT_sb[:, c, n * 128 : (n + 1) * 128],
                                        rhs=wv_sb[:, c, h * 96 : (h + 1) * 96],
                                        start=(c == 0),
                                        stop=(c == NCC - 1),
                                    )
                            for half in range(2):
                                nc.vector.tensor_copy(
                                    out=vp_sb[h][:, 2 * n2 + half, 0:96],
                                    in_=vacc[:, half * 512 : half * 512 + 96],
                                )

                        return emit

                    # ordered by xT n-quarter arrival: [q,k,v*4] per quarter
                    for n2 in range(N // 1024):
                        units.append(qk_unit(wq_sb, qT_sb[h], n2))
                        units.append(qk_unit(wk_sb, kT_sb[h], n2))
                        for nv in range(4 * n2, 4 * n2 + 4):
                            units.append(v_unit(nv))
                    return units

                # ---------------- output combine (quartered RS) ----------------
                y_bounce = dram.tile([N, C], BF16, tag="ybounce")
                HR = N // 4  # rows per RS quarter (1024)
                SH = HR // 4  # rows per rank per quarter (256)
                rs_outs = [None] * 4

                def emit_rs(k):
                    rs_out = dram.tile([SH, C], BF16, tag="rsout")
                    if _DEBUG_NO_RS:
                        nc.sync.dma_start(rs_out[:, :], y_bounce[k * HR : k * HR + SH, :])
                    else:
                        nc.gpsimd.collective_compute(
                            "ReduceScatter",
                            mybir.AluOpType.add,
                            replica_groups=GROUPS,
                            ins=[y_bounce[k * HR : (k + 1) * HR, :].opt()],
                            outs=[rs_out.opt()],
                        )
                    rs_outs[k] = rs_out

                def emit_epilogue(k):
                    rt = ybp.tile([128, SH // 128, C], BF16, tag="rsb", bufs=2)
                    nc.sync.dma_start(
                        rt, rs_outs[k].rearrange("(o p) m -> p o m", p=128)
                    )
                    rtf = ybp.tile([128, SH // 128, C], F32, tag="rsf", bufs=2)
                    nc.vector.tensor_copy(out=rtf, in_=rt)
                    nc.gpsimd.dma_start(
                        out_ext[k * SH : (k + 1) * SH, :].rearrange(
                            "(o p) m -> p o m", p=128
                        ),
                        rtf,
                        accum_op=mybir.AluOpType.add,
                    )

                def attention(h, unit_plan, carry_in=None):
                    """Flash attention for head h over all q-ranges; emits
                    closures from unit_plan[(qr, kcp)] between score groups
                    (QKV units placed just-in-time before the kcp that needs
                    their output). Each q-range's finalize (normalize/
                    transpose/proj) is deferred into the next q-range's score
                    stream so the PE priority order keeps scores (which feed
                    the ACT engine) ahead of proj/epilogue work."""
                    unit_plan = unit_plan or {}
                    pending = [carry_in]
                    for qr in range(N_QR):
                        # all 4 q128-chunk accumulators share ONE psum bank;
                        # the first matmul's start=True zeroes the whole 2KB
                        # zero-region, later j's first matmuls ride on it
                        o4 = op.tile([128, 4, 97], F32, tag="o")
                        for kcp in range(N_KC // 2):
                            sp = ps.tile([128, 2, 512], F32, tag="ps")
                            for kk in range(2):
                                kc = 2 * kcp + kk
                                nc.tensor.matmul(
                                    sp[:, kk, :],
                                    lhsT=kT_sb[h][:, kc * 128 : (kc + 1) * 128],
                                    rhs=qT_sb[h][:, qr * QR : (qr + 1) * QR],
                                    start=True,
                                    stop=True,
                                )
                            p_t = pp.tile([128, 2, 512], BF16, tag="p")
                            nc.scalar.activation(
                                p_t, sp, mybir.ActivationFunctionType.Exp
                            )
                            for kk in range(2):
                                kc = 2 * kcp + kk
                                for j in range(4):
                                    nc.tensor.matmul(
                                        o4[:, j, 0:97],
                                        lhsT=p_t[:, kk, j * 128 : (j + 1) * 128],
                                        rhs=vp_sb[h][:, kc, :],
                                        start=(kcp == 0 and kk == 0 and j == 0),
                                        stop=(kcp == N_KC // 2 - 1 and kk == 1),
                                        skip_group_check=True,
                                    )
                            if kcp == 2 and pending[0] is not None:
                                pending[0]()
                                pending[0] = None
                            for u in unit_plan.pop((qr, kcp), ()):
                                u()

                        def finalize(qr, o4):
                            # normalize per-partition: on = o[:,:96] / o[:,96]
                            # (d padded to 128 for XBAR alignment; pad zeroed)
                            on_qr = onp.tile([128, 4, 128], BF16, tag="on")
                            nc.vector.memset(on_qr[:, :, 96:128], 0.0)
                            r4 = misc.tile([128, 4, 1], F32, tag="r")
                            nc.vector.reciprocal(r4, o4[:, :, 96:97])
                            r4_ap = r4[:, :, :]
                            r4_bc = bass.AP(
                                tensor=r4_ap.tensor,
                                offset=r4_ap.offset,
                                ap=list(r4_ap.ap[:-1]) + [[0, 96]],
                            )
                            nc.vector.tensor_tensor(
                                on_qr[:, :, 0:96],
                                o4[:, :, 0:96],
                                r4_bc,
                                mybir.AluOpType.mult,
                            )
                            # oT via DMA XBAR (SP queue; dispatch is
                            # async from the transfer, and proj is already a
                            # q-range behind via the deferred finalize)
                            nc.sync.dma_start_transpose(
                                onT_sb[h][:, qr * 4 : (qr + 1) * 4, :], on_qr
                            )

                            if h == 1:
                                # projection: y[n,:] = sum_h onT_h[:, n]^T @ wp_h
                                for ns in range(QR // 128):
                                    nchunk = qr * 4 + ns
                                    yp = ps.tile([128, 1024], F32, tag="ps")
                                    for hh in range(2):
                                        for lo, hi in [(0, 512), (512, 768)]:
                                            nc.tensor.matmul(
                                                yp[:, lo:hi],
                                                lhsT=onT_sb[hh][0:96, nchunk, :],
                                                rhs=wp_sb[:96, hh, lo:hi],
                                                start=(hh == 0),
                                                stop=(hh == 1),
                                            )
                                    y_sb = ybp.tile([128, C], BF16, tag="y")
                                    nc.vector.tensor_copy(out=y_sb, in_=yp[:, :C])
                                    # SP ring: bulk y writes (transposes
                                    # live on the ACT queue, collectives on
                                    # Pool, so nothing latency-critical queues
                                    # behind these)
                                    nc.sync.dma_start(
                                        y_bounce[
                                            nchunk * 128 : (nchunk + 1) * 128, :
                                        ],
                                        y_sb,
                                    )
                                # fire a quarter ReduceScatter every 1024 rows
                                # (each overlaps the remaining compute; only
                                # the last quarter's RS+epilogue is exposed)
                                if qr % 2 == 1:
                                    k = qr // 2
                                    emit_rs(k)
                                    emit_epilogue(k)

                        from functools import partial

                        pending[0] = partial(finalize, qr, o4)
                    return pending[0]

                # head-0 QKV emitted just-in-time inside head-0's own
                # attention sweep: v_n lands right before the kcp that
                # consumes it, k_n before its first score group, so the exp
                # stream starts after only [q0, k0, v0] (~5us of PE) instead
                # of the full head QKV (~25us)
                u0 = qkv_units(0)
                q_un = [u0[6 * i] for i in range(4)]
                k_un = [u0[6 * i + 1] for i in range(4)]
                v_un = [u for i in range(4) for u in u0[6 * i + 2 : 6 * i + 6]]
                plan = {}

                def put(qr, kcp, u):
                    plan.setdefault((qr, kcp), []).append(u)

                for n in range(15):
                    put(0, n, v_un[n + 1])
                put(0, 2, k_un[1])
                put(0, 6, k_un[2])
                put(0, 10, k_un[3])
                for i in range(1, 4):
                    put(i, 5, q_un[i])
                u1 = qkv_units(1)
                if _INTERLEAVE_QKV1:
                    slots = [
                        (qr, kcp) for qr in range(1, 8) for kcp in (3, 7, 11, 14)
                    ]
                    assert len(slots) >= len(u1)
                    for u, s in zip(u1, slots):
                        put(s[0], s[1], u)
                    u1 = []
                for u in (q_un[0], k_un[0], v_un[0], *u1):
                    u()
                # head 0's last finalize is carried into head 1's stream;
                # head 1's last finalize (proj tail + final RS + epilogues)
                # fires after the loop
                carry = attention(0, plan)
                attention(1, None, carry_in=carry)()

    _hoist_waits(nc)
    return nc


_NC_CACHE = None


def _get_nc():
    global _NC_CACHE
    if _NC_CACHE is None:
        _NC_CACHE = build()
    return _NC_CACHE


def make_in_maps(x, w_qkv, w_proj, b_proj):
    x = np.asarray(x, dtype=np.float32)
    w_qkv = np.asarray(w_qkv, dtype=np.float32).reshape(C, 3, H, DH)
    w_proj = np.asarray(w_proj, dtype=np.float32)
    b_proj = np.asarray(b_proj, dtype=np.float32)
    scale = DH ** -0.5

    xT_b = [
        np.ascontiguousarray(x[b].T).astype(BF16_NP) for b in range(B)
    ]  # [C, N] each
    in_maps = []
    for i in range(N_CORES):
        b = i // 4
        h0 = HEADS_PER_CORE * (i % 4)
        sl = slice(h0, h0 + HEADS_PER_CORE)
        wq_i = (w_qkv[:, 0, sl, :].reshape(C, HD) * scale).astype(BF16_NP)
        wk_i = w_qkv[:, 1, sl, :].reshape(C, HD).astype(BF16_NP)
        wv_i = w_qkv[:, 2, sl, :].reshape(C, HD).astype(BF16_NP)
        wp_i = np.ascontiguousarray(
            w_proj.reshape(H, DH, C)[sl].reshape(HD, C)
        ).astype(BF16_NP)
        in_maps.append(
            {
                "xT": xT_b[b],
                "wq": np.ascontiguousarray(wq_i),
                "wk": np.ascontiguousarray(wk_i),
                "wv": np.ascontiguousarray(wv_i),
                "wp": wp_i,
                "bias": b_proj,
            }
        )
    return in_maps


def assemble(results):
    # rank r of batch-group b holds, per RS quarter k, global rows
    # [k*1024 + r*256, k*1024 + (r+1)*256) in out_ext rows [k*256, (k+1)*256)
    out = np.empty((B, N, C), dtype=np.float32)
    SH = N // 16
    for i in range(N_CORES):
        b, r = i // 4, i % 4
        shard = results[i]["out"]
        for k in range(4):
            lo = k * (N // 4) + r * SH
            out[b, lo : lo + SH, :] = shard[k * SH : (k + 1) * SH]
    return out


def kernel(x, w_qkv, w_proj, b_proj):
    nc = _get_nc()
    in_maps = make_in_maps(x, w_qkv, w_proj, b_proj)
    res = run_bass_kernel_spmd(nc, in_maps, core_ids=list(range(N_CORES)))
    return assemble(res.results)
